# revision 19
# baseline (speedup 1.0000x reference)
"""Trainium2 Bass kernel for the BalSCL/SSL balanced supervised-contrastive loss.

Distribution: data-parallel over the 8192 anchor rows, 1024 rows per core on
8 NeuronCores.  Each core returns two partial-loss scalars (the conf-weighted
sum of ln S_i over its two 512-row chunks); the host combines them with the
host-computed linear (mean-positive-logit) term and conf denominator.

Math (restructured from the reference, analytically identical):
  N = 8292 columns (8192 anchors + 100 class centers), all unit-norm.
  The row-max subtraction in the reference cancels analytically, so
    loss_i = ln(S_i) - (10/m_i) * Sm_i
  with
    S_i  = sum_{j != i} exp(10 * f_i . g_j) / (cc_j - [lab_j == lab_i])
    Sm_i = sum_{j != i, lab_j == lab_i} f_i . g_j      (host, exact f64)
    m_i  = cc[lab_i] - 1
  Device work per core: raw logits r = fTg.T @ fTc (bf16 PE), elementwise
  exp(10 r) quantized to fp8e5m2, and per-class sums E[c,i] via fp8 DoubleRow
  matmuls (two 128-row j-tiles per PE pass).  S_i = sum_c W2c[c,i] E[c,i] - dg_i
  where W2c folds the per-class balanced weights and the conf mask, and dg
  subtracts the diagonal (j == i) fp8 term bit-exactly.

  The exp work is split between the Scalar engine (true spline exp, RNE to
  fp8e5m2 -- hardware-validated exact) and the Vector engine (Schraudolph
  trick: y = round(r*40/ln2 + B) as int8, bit-reinterpreted as fp8e5m2; B
  calibrated so the mean log error over the logit distribution vanishes).
  Pair p of j-tiles goes to Scalar iff p % 4 < 2, so for every core the
  chunk-0 diagonal lands in a Scalar pair and the chunk-1 diagonal in a
  Vector pair; dg uses the matching generator per chunk.
"""

import os
import sys

sys.path.insert(0, "/opt/trn_rl_repo")

import numpy as np
import ml_dtypes

import concourse.bass as bass  # noqa: F401
import concourse.bacc as bacc
import concourse.tile as tile
from concourse import mybir
from concourse.bass_utils import run_bass_kernel_spmd

F32 = mybir.dt.float32
BF16 = mybir.dt.bfloat16
FP8 = mybir.dt.float8e5
I8 = mybir.dt.int8
BF = ml_dtypes.bfloat16
F8NP = ml_dtypes.float8_e5m2
AF = mybir.ActivationFunctionType
ALU = mybir.AluOpType
DR = mybir.MatmulPerfMode.DoubleRow

B2, C, D = 8192, 100, 128
TEMP = 0.1
N = B2 + C
TILES = 66                 # 65 real j-tiles + 1 zero pad (for pairing)
PAIRS = TILES // 2         # 33
NPAD2 = TILES * 128        # 8448
CP = 112                   # padded class count (fp8 pair stride % 16 == 0)
CORES = 8
R = B2 // CORES            # 1024 rows per core
CH = 512                   # i-chunk width (one fp32 PSUM bank)
A_TRICK = 40.0 / np.log(2.0)   # 57.70780163555855
B_TRICK = 59.8                 # calibrated: zero mean log-error (see sim)
N_WARM = 4

_NC_CACHE = {}

# Combined exp+ln activation-table set: a single ACT_TABLE_LOAD.
_orig_gat = bacc.get_activation_tables


def _gat_combined(arch):
    tabs = _orig_gat(arch)
    out = {}
    for name, funcs in tabs.items():
        if name in ("exp_and_others", "exp_and_friends", "natural_log"):
            out[name] = set()  # keep position (set ids are positional)
        else:
            out[name] = funcs
    return out


def _is_act(p):
    return p % 2 == 0


def _build_nc():
    bacc.get_activation_tables = _gat_combined
    try:
        return _build_nc_inner()
    finally:
        bacc.get_activation_tables = _orig_gat


def _build_nc_inner():
    nc = bacc.Bacc()

    fTg = nc.dram_tensor("fTg", [D, NPAD2], BF16, kind="ExternalInput")
    TAg = nc.dram_tensor("TAg", [128, TILES * CP], FP8, kind="ExternalInput")
    fTc = nc.dram_tensor("fTc", [D, R], BF16, kind="ExternalInput")
    W2c = nc.dram_tensor("W2c", [C, R], BF16, kind="ExternalInput")
    mc = nc.dram_tensor("mc", [1, R], F32, kind="ExternalInput")
    cm1 = nc.dram_tensor("cm1", [1, R], F32, kind="ExternalInput")
    outd = nc.dram_tensor("out", [1, 2], F32, kind="ExternalOutput")

    with tile.TileContext(nc) as tc:
        with (
            tc.tile_pool(name="consts", bufs=1) as cp,
            tc.tile_pool(name="expp", bufs=5) as ep,
            tc.tile_pool(name="rawp", bufs=3, space="PSUM") as rp,
            tc.tile_pool(name="epsp", bufs=1, space="PSUM") as pp,
            tc.tile_pool(name="smp", bufs=1, space="PSUM") as sp,
        ):
            # ---------------- input loads (sync queue: big streams) --------
            s_fTc = cp.tile([D, R], BF16)
            s_fTg = cp.tile([D, NPAD2], BF16)
            s_TAg = cp.tile([128, TILES * CP], FP8)
            nc.sync.dma_start(out=s_fTc[:, 0:CH], in_=fTc[:, 0:CH])
            nc.sync.dma_start(out=s_fTg[:, 0:512], in_=fTg[:, 0:512])
            nc.sync.dma_start(out=s_fTg[:, 512:1536], in_=fTg[:, 512:1536])
            nc.sync.dma_start(out=s_fTc[:, CH:R], in_=fTc[:, CH:R])
            nc.sync.dma_start(out=s_fTg[:, 1536:4224], in_=fTg[:, 1536:4224])
            nc.sync.dma_start(out=s_fTg[:, 4224:NPAD2], in_=fTg[:, 4224:NPAD2])

            # gpsimd queue: memsets + small/medium loads
            s_scr = cp.tile([128, CH], BF16)
            nc.gpsimd.memset(s_scr, 1.0)
            s_ones = cp.tile([128, 1], F32)
            nc.gpsimd.memset(s_ones, 1.0)
            s_ones_bf = cp.tile([128, 1], BF16)
            nc.gpsimd.memset(s_ones_bf, 1.0)
            s_mc = cp.tile([1, R], F32)
            nc.gpsimd.dma_start(out=s_mc, in_=mc[:])
            s_cm1 = cp.tile([1, R], F32)
            nc.gpsimd.dma_start(out=s_cm1, in_=cm1[:])
            nc.gpsimd.dma_start(out=s_TAg[:, 0:448], in_=TAg[:, 0:448])
            nc.gpsimd.dma_start(out=s_TAg[:, 448:2240], in_=TAg[:, 448:2240])
            nc.gpsimd.dma_start(
                out=s_TAg[:, 2240 : TILES * CP], in_=TAg[:, 2240 : TILES * CP]
            )
            s_W2c = cp.tile([C, R], BF16)
            nc.gpsimd.dma_start(out=s_W2c, in_=W2c[:])

            # ---------------- PE warm-up (HAM un-throttle) -----------------
            warmPS = sp.tile([128, CH], F32, name="warmPS", tag="sm")
            for _ in range(N_WARM):
                nc.tensor.matmul(
                    warmPS, lhsT=s_scr[:, 0:128], rhs=s_scr, start=True, stop=True
                )

            # ---------------- fsq / ed / e1 smalls (early) -----------------
            # sq_k on Vector (f32 exact squares of the bf16 features)
            sq_t = []
            for k in (0, 1):
                sq = cp.tile([128, CH], F32, name=f"sq{k}", tag=f"sq{k}")
                nc.vector.tensor_mul(
                    sq, s_fTc[:, k * CH : (k + 1) * CH], s_fTc[:, k * CH : (k + 1) * CH]
                )
                sq_t.append(sq)

            ed_t = [None, None]   # fp8e5 diag exp per chunk
            e1_t = [None, None]   # (dg+1)*conf - 1 per chunk
            fsqPS_t = [None, None]

            s_Sall = cp.tile([1, R], F32)
            outsb = cp.tile([1, 2], F32)

            def mk_fsq(k):
                fsqPS = sp.tile([1, CH], F32, name=f"fsqPS{k}", tag="sm")
                nc.tensor.matmul(fsqPS, lhsT=s_ones, rhs=sq_t[k], start=True, stop=True)
                fsqPS_t[k] = fsqPS

            # chunk-k diagonal: cols [0:256] live in an even (Scalar) pair,
            # cols [256:512] in an odd (Vector) pair, for every core.
            HH = CH // 2

            def mk_ed_act(k):
                if ed_t[k] is None:
                    ed_t[k] = cp.tile([1, CH], FP8, name=f"ed{k}", tag=f"ed{k}")
                nc.scalar.activation(
                    out=ed_t[k][:, 0:HH], in_=fsqPS_t[k][:, 0:HH],
                    func=AF.Exp, scale=1.0 / TEMP,
                )

            def mk_ed_dve(k):
                if ed_t[k] is None:
                    ed_t[k] = cp.tile([1, CH], FP8, name=f"ed{k}", tag=f"ed{k}")
                nc.vector.tensor_scalar(
                    out=ed_t[k][:, HH:CH].bitcast(I8), in0=fsqPS_t[k][:, HH:CH],
                    scalar1=A_TRICK, scalar2=B_TRICK, op0=ALU.mult, op1=ALU.add,
                )

            def mk_e1(k, step):
                # e1 = ed * (minv*conf) + (conf - 1); mc/cm1 folded on host
                i0 = k * CH
                if step == 0:
                    t = cp.tile([1, CH], F32, name=f"dgt{k}", tag=f"dgt{k}")
                    nc.vector.tensor_mul(t, ed_t[k], s_mc[:, i0 : i0 + CH])
                    e1_t[k] = t
                else:
                    t2 = cp.tile([1, CH], F32, name=f"e1{k}", tag=f"e1{k}")
                    nc.vector.tensor_add(t2, e1_t[k], s_cm1[:, i0 : i0 + CH])
                    e1_t[k] = t2

            # ---------------- main pipeline --------------------------------
            exps_t = {}

            def raw_pair(k, p):
                rawPS = rp.tile([128, 2 * CH], F32, name=f"raw{k}_{p}", tag="raw")
                for q in (0, 1):
                    t = 2 * p + q
                    nc.tensor.matmul(
                        rawPS[:, q * CH : (q + 1) * CH],
                        lhsT=s_fTg[:, 128 * t : 128 * (t + 1)],
                        rhs=s_fTc[:, k * CH : (k + 1) * CH],
                        start=True,
                        stop=True,
                    )
                return rawPS

            def exp_pair(k, p, rawPS):
                exps = ep.tile([128, 2 * CH], FP8, name=f"exps{k}_{p}", tag="exps")
                if _is_act(p):
                    nc.scalar.activation(
                        out=exps, in_=rawPS, func=AF.Exp, scale=1.0 / TEMP
                    )
                else:
                    # two half-ops: frees the first rawPS bank ~550ns sooner
                    for h in (0, 1):
                        nc.vector.tensor_scalar(
                            out=exps[:, h * CH : (h + 1) * CH].bitcast(I8),
                            in0=rawPS[:, h * CH : (h + 1) * CH],
                            scalar1=A_TRICK, scalar2=B_TRICK,
                            op0=ALU.mult, op1=ALU.add,
                        )
                exps_t[(k, p)] = exps

            EPS_t = [None, None]

            def e_mm(k, p):
                if EPS_t[k] is None:
                    EPS_t[k] = pp.tile([CP, CH], F32, name=f"EPS{k}", tag="EPS")
                exps = exps_t.pop((k, p))
                nc.tensor.matmul(
                    EPS_t[k],
                    lhsT=s_TAg[:, 224 * p : 224 * (p + 1)].rearrange(
                        "a (two c) -> a two c", two=2
                    ),
                    rhs=exps[:].rearrange("a (two n) -> a two n", two=2),
                    start=(p == 0),
                    stop=(p == PAIRS - 1),
                    perf_mode=DR,
                )

            srowPS_t = [None, None]

            def mk_w2e(k):
                i0 = k * CH
                w2e = cp.tile([C, CH], BF16, name=f"W2E{k}", tag=f"W2E{k}")
                nc.vector.tensor_mul(w2e, EPS_t[k][0:C, :], s_W2c[:, i0 : i0 + CH])
                return w2e

            def mk_srow(k, w2e):
                srowPS = sp.tile([1, CH], F32, name=f"srowPS{k}", tag="sm")
                nc.tensor.matmul(
                    srowPS, lhsT=s_ones_bf[0:C, :], rhs=w2e, start=True, stop=True
                )
                srowPS_t[k] = srowPS

            def mk_sall(k):
                nc.vector.tensor_sub(
                    s_Sall[:, k * CH : (k + 1) * CH], srowPS_t[k], e1_t[k]
                )

            lg_t = [None, None]

            def mk_ln(k):
                lg = cp.tile([1, CH], F32, name=f"lg{k}", tag=f"lg{k}")
                nc.scalar.activation(
                    out=lg, in_=s_Sall[:, k * CH : (k + 1) * CH], func=AF.Ln,
                    accum_out=outsb[:, k : k + 1],
                )
                lg_t[k] = lg

            # Vector-queue side-work scheduled after specific DVE exp pairs:
            vec_after = {
                (0, 11): [lambda: mk_ed_dve(0)],
                (0, 15): [lambda: mk_ed_dve(1)],
                (0, 19): [lambda: mk_e1(0, 0)],
                (0, 21): [lambda: mk_e1(0, 1)],
                (0, 25): [lambda: mk_e1(1, 0)],
                (0, 27): [lambda: mk_e1(1, 1)],
                (1, 6): [lambda: mk_sall(0)],
            }
            # Scalar-queue side-work
            sca_after = {
                (0, 10): [lambda: mk_ed_act(0)],
                (0, 14): [lambda: mk_ed_act(1)],
                (1, 9): [lambda: mk_ln(0)],
            }

            for k in (0, 1):
                raw_t = {}
                for p in range(PAIRS):
                    raw_t[p] = raw_pair(k, p)
                    if k == 0 and p == 8:
                        mk_fsq(0)
                    if k == 0 and p == 12:
                        mk_fsq(1)
                    if k == 1 and p == 3:
                        # chunk-0 tail reduction once W2E0 is ready
                        mk_srow(0, w2e0)
                    exp_pair(k, p, raw_t.pop(p))
                    for fn in sca_after.pop((k, p), ()):
                        fn()
                    for fn in vec_after.pop((k, p), ()):
                        fn()
                    if p >= 2:
                        e_mm(k, p - 2)
                e_mm(k, PAIRS - 2)
                e_mm(k, PAIRS - 1)
                if k == 0:
                    w2e0 = mk_w2e(0)
                else:
                    w2e1 = mk_w2e(1)
                    mk_srow(1, w2e1)

            # ---------------- tail: ln + accumulate ------------------------
            mk_sall(1)
            mk_ln(1)
            nc.sync.dma_start(out=outd[:], in_=outsb)

    nc.finalize()
    return nc


def _get_nc():
    if "nc" not in _NC_CACHE:
        _NC_CACHE["nc"] = _build_nc()
    return _NC_CACHE["nc"]


def _prep_inputs(centers1, features, targets, conf_mask):
    f32 = np.float32
    features = np.ascontiguousarray(features, dtype=f32)
    centers1 = np.ascontiguousarray(centers1, dtype=f32).reshape(-1, D)
    targets = np.ascontiguousarray(targets, dtype=f32)
    conf_mask = np.ascontiguousarray(conf_mask, dtype=f32)

    feats_all = np.concatenate([features, centers1], axis=0)  # [N, D]
    fa_pad = np.zeros((NPAD2, D), dtype=f32)
    fa_pad[:N] = feats_all
    fTg_np = np.ascontiguousarray(fa_pad.T).astype(BF)  # [D, NPAD2]

    TA_pad = np.zeros((NPAD2, CP), dtype=f32)
    TA_pad[:B2, :C] = targets
    TA_pad[B2 : B2 + C, :C] = np.eye(C, dtype=f32)
    TAg_np = np.ascontiguousarray(
        TA_pad.reshape(TILES, 128, CP).transpose(1, 0, 2).reshape(128, TILES * CP)
    ).astype(F8NP)

    labels = targets.argmax(axis=1)
    cc = targets.sum(axis=0, dtype=np.float64) + 1.0  # [C]
    mpos = np.maximum(cc - 1.0, 1.0)
    W2 = np.where(
        targets.T == 1.0, 1.0 / mpos[:, None], 1.0 / cc[:, None]
    )  # [C, B2] f64
    minv_all = (1.0 / mpos[labels]).astype(f32)  # [B2]

    # host linear term: exact f32-feature positive-pair mean logits
    gsum = np.zeros((C, D), dtype=np.float64)
    np.add.at(gsum, labels, features.astype(np.float64))
    gsum += centers1.astype(np.float64)  # class centers are their own class
    feats64 = features.astype(np.float64)
    Sm = (feats64 * gsum[labels]).sum(axis=1) - (feats64 * feats64).sum(axis=1)
    conf64 = conf_mask.astype(np.float64)
    numB = float((conf64 * (1.0 / TEMP) * Sm / mpos[labels]).sum())
    den = float(conf64.sum())

    in_maps = []
    for c in range(CORES):
        rows = slice(c * R, (c + 1) * R)
        fTc_np = np.ascontiguousarray(fTg_np[:, c * R : (c + 1) * R])
        W2c_np = np.ascontiguousarray(
            (W2[:, rows] * conf64[None, rows]).astype(f32)
        ).astype(BF)
        in_maps.append(
            {
                "fTg": fTg_np,
                "TAg": TAg_np,
                "fTc": fTc_np,
                "W2c": W2c_np,
                "mc": np.ascontiguousarray(
                    (minv_all[rows] * conf_mask[rows]).reshape(1, R)
                ),
                "cm1": np.ascontiguousarray(
                    (conf_mask[rows] - 1.0).reshape(1, R)
                ),
            }
        )
    return in_maps, numB, den


def _run(centers1, features, targets, conf_mask, trace=False, trace_cores=None):
    in_maps, numB, den = _prep_inputs(centers1, features, targets, conf_mask)
    nc = _get_nc()
    kwargs = {}
    if trace:
        # NTFF profiling under axon: shim the (absent) antenv.axon_hooks
        # module and skip the artifact bucket upload.
        import types
        import concourse.bass_utils as bass_utils

        if "antenv.axon_hooks" not in sys.modules:
            mod = types.ModuleType("antenv.axon_hooks")
            mod._hook = None

            def set_axon_ntff_profile_hook(h):
                mod._hook = h

            def get_axon_ntff_profile_hook():
                return mod._hook

            mod.set_axon_ntff_profile_hook = set_axon_ntff_profile_hook
            mod.get_axon_ntff_profile_hook = get_axon_ntff_profile_hook
            sys.modules["antenv.axon_hooks"] = mod
            from trn_agent_boot.trn_boot import _ntff_profile_via_ctypes

            set_axon_ntff_profile_hook(
                _ntff_profile_via_ctypes("/opt/axon/libaxon_pjrt.so")
            )
        bass_utils.upload_artifacts = lambda tmpdir: "local://" + tmpdir
        kwargs = {"trace": True}
        if trace_cores is not None:
            kwargs["trace_cores"] = trace_cores
    res = run_bass_kernel_spmd(nc, in_maps, core_ids=list(range(CORES)), **kwargs)
    numA = 0.0
    for r in res.results:
        numA += float(r["out"][0, 0]) + float(r["out"][0, 1])
    loss = np.array((numA - numB) / den, dtype=np.float32)
    return loss, res


def kernel(centers1, features, targets, cls_num_list, conf_mask):
    loss, _ = _run(centers1, features, targets, conf_mask)
    return loss


# revision 22
# speedup vs baseline: 1.0495x; 1.0495x over previous
"""Trainium2 Bass kernel for the BalSCL/SSL balanced supervised-contrastive loss.

Distribution: data-parallel over the 8192 anchor rows, 1024 rows per core on
8 NeuronCores.  Each core returns two partial-loss scalars (the conf-weighted
sum of ln S_i over its two 512-row chunks); the host combines them with the
host-computed linear (mean-positive-logit) term and conf denominator.

Math (restructured from the reference, analytically identical):
  N = 8292 columns (8192 anchors + 100 class centers), all unit-norm.
  The row-max subtraction in the reference cancels analytically, so
    loss_i = ln(S_i) - (10/m_i) * Sm_i
  with
    S_i  = sum_{j != i} exp(10 * f_i . g_j) / (cc_j - [lab_j == lab_i])
    Sm_i = sum_{j != i, lab_j == lab_i} f_i . g_j      (host, exact f64)
    m_i  = cc[lab_i] - 1
  Device work per core: raw logits r = fTg.T @ fTc (bf16 PE), elementwise
  exp(10 r) quantized to fp8e5m2, and per-class sums E[c,i] via fp8 DoubleRow
  matmuls (two 128-row j-tiles per PE pass).  S_i = sum_c W2c[c,i] E[c,i] - dg_i
  where W2c folds the per-class balanced weights and the conf mask, and dg
  subtracts the diagonal (j == i) fp8 term bit-exactly.

  The exp work is split between the Scalar engine (true spline exp, RNE to
  fp8e5m2 -- hardware-validated exact) and the Vector engine (Schraudolph
  trick: y = round(r*40/ln2 + B) as int8, bit-reinterpreted as fp8e5m2; B
  calibrated so the mean log error over the logit distribution vanishes).
  Pair p of j-tiles goes to Scalar iff p % 4 < 2, so for every core the
  chunk-0 diagonal lands in a Scalar pair and the chunk-1 diagonal in a
  Vector pair; dg uses the matching generator per chunk.
"""

import os
import sys

sys.path.insert(0, "/opt/trn_rl_repo")

import numpy as np
import ml_dtypes

import concourse.bass as bass  # noqa: F401
import concourse.bacc as bacc
import concourse.tile as tile
from concourse import mybir
from concourse.bass_utils import run_bass_kernel_spmd

F32 = mybir.dt.float32
BF16 = mybir.dt.bfloat16
FP8 = mybir.dt.float8e5
I8 = mybir.dt.int8
BF = ml_dtypes.bfloat16
F8NP = ml_dtypes.float8_e5m2
AF = mybir.ActivationFunctionType
ALU = mybir.AluOpType
DR = mybir.MatmulPerfMode.DoubleRow

B2, C, D = 8192, 100, 128
TEMP = 0.1
N = B2 + C
TILES = 66                 # 65 real j-tiles + 1 zero pad (for pairing)
PAIRS = TILES // 2         # 33
NPAD2 = TILES * 128        # 8448
CP = 112                   # padded class count (fp8 pair stride % 16 == 0)
CORES = 8
R = B2 // CORES            # 1024 rows per core
CH = 512                   # i-chunk width (one fp32 PSUM bank)
A_TRICK = 40.0 / np.log(2.0)   # 57.70780163555855
B_TRICK = 59.8                 # calibrated: zero mean log-error (see sim)
N_WARM = 4

_NC_CACHE = {}

# Combined exp+ln activation-table set: a single ACT_TABLE_LOAD.
_orig_gat = bacc.get_activation_tables


def _gat_combined(arch):
    tabs = _orig_gat(arch)
    out = {}
    for name, funcs in tabs.items():
        if name in ("exp_and_others", "exp_and_friends", "natural_log"):
            out[name] = set()  # keep position (set ids are positional)
        else:
            out[name] = funcs
    return out


def _is_act(p):
    return p % 2 == 0


def _build_nc():
    bacc.get_activation_tables = _gat_combined
    try:
        return _build_nc_inner()
    finally:
        bacc.get_activation_tables = _orig_gat


def _build_nc_inner():
    nc = bacc.Bacc()

    fTg = nc.dram_tensor("fTg", [D, NPAD2], BF16, kind="ExternalInput")
    TAg = nc.dram_tensor("TAg", [128, TILES * CP], FP8, kind="ExternalInput")
    fTc = nc.dram_tensor("fTc", [D, R], BF16, kind="ExternalInput")
    W2c = nc.dram_tensor("W2c", [C, R], BF16, kind="ExternalInput")
    mc = nc.dram_tensor("mc", [1, R], F32, kind="ExternalInput")
    cm1 = nc.dram_tensor("cm1", [1, R], F32, kind="ExternalInput")
    outd = nc.dram_tensor("out", [1, 2], F32, kind="ExternalOutput")

    with tile.TileContext(nc) as tc:
        with (
            tc.tile_pool(name="consts", bufs=1) as cp,
            tc.tile_pool(name="expp", bufs=5) as ep,
            tc.tile_pool(name="rawp", bufs=3, space="PSUM") as rp,
            tc.tile_pool(name="epsp", bufs=1, space="PSUM") as pp,
            tc.tile_pool(name="smp", bufs=1, space="PSUM") as sp,
        ):
            # ---------------- input loads (sync queue: big streams) --------
            s_fTc = cp.tile([D, R], BF16)
            s_fTg = cp.tile([D, NPAD2], BF16)
            s_TAg = cp.tile([128, TILES * CP], FP8)
            nc.sync.dma_start(out=s_fTc[:, 0:CH], in_=fTc[:, 0:CH])
            nc.sync.dma_start(out=s_fTg[:, 0:256], in_=fTg[:, 0:256])
            nc.sync.dma_start(out=s_fTg[:, 256:1024], in_=fTg[:, 256:1024])
            nc.sync.dma_start(out=s_fTc[:, CH:R], in_=fTc[:, CH:R])
            nc.sync.dma_start(out=s_fTg[:, 1024:2560], in_=fTg[:, 1024:2560])
            nc.sync.dma_start(out=s_fTg[:, 2560:5376], in_=fTg[:, 2560:5376])
            nc.sync.dma_start(out=s_fTg[:, 5376:NPAD2], in_=fTg[:, 5376:NPAD2])

            # gpsimd queue: memsets + small/medium loads
            s_scr = cp.tile([128, CH], BF16)
            nc.gpsimd.memset(s_scr, 1.0)
            s_ones = cp.tile([128, 1], F32)
            nc.gpsimd.memset(s_ones, 1.0)
            s_ones_bf = cp.tile([128, 1], BF16)
            nc.gpsimd.memset(s_ones_bf, 1.0)
            s_mc = cp.tile([1, R], F32)
            nc.gpsimd.dma_start(out=s_mc, in_=mc[:])
            s_cm1 = cp.tile([1, R], F32)
            nc.gpsimd.dma_start(out=s_cm1, in_=cm1[:])
            nc.gpsimd.dma_start(out=s_TAg[:, 0:448], in_=TAg[:, 0:448])
            nc.gpsimd.dma_start(out=s_TAg[:, 448:2240], in_=TAg[:, 448:2240])
            nc.gpsimd.dma_start(
                out=s_TAg[:, 2240 : TILES * CP], in_=TAg[:, 2240 : TILES * CP]
            )
            s_W2c = cp.tile([C, R], BF16)
            nc.gpsimd.dma_start(out=s_W2c, in_=W2c[:])

            # ---------------- PE warm-up (HAM un-throttle) -----------------
            warmPS = sp.tile([128, CH], F32, name="warmPS", tag="sm")
            for _ in range(N_WARM):
                nc.tensor.matmul(
                    warmPS, lhsT=s_scr[:, 0:128], rhs=s_scr, start=True, stop=True
                )

            # ---------------- fsq / ed / e1 smalls (early) -----------------
            # sq_k on Vector (f32 exact squares of the bf16 features)
            sq_t = []
            for k in (0, 1):
                sq = cp.tile([128, CH], F32, name=f"sq{k}", tag=f"sq{k}")
                nc.vector.tensor_mul(
                    sq, s_fTc[:, k * CH : (k + 1) * CH], s_fTc[:, k * CH : (k + 1) * CH]
                )
                sq_t.append(sq)

            ed_t = [None, None]   # fp8e5 diag exp per chunk
            e1_t = [None, None]   # (dg+1)*conf - 1 per chunk
            fsqPS_t = [None, None]

            s_Sall = cp.tile([1, R], F32)
            outsb = cp.tile([1, 2], F32)

            def mk_fsq(k):
                fsqPS = sp.tile([1, CH], F32, name=f"fsqPS{k}", tag="sm")
                nc.tensor.matmul(fsqPS, lhsT=s_ones, rhs=sq_t[k], start=True, stop=True)
                fsqPS_t[k] = fsqPS

            # chunk-k diagonal: cols [0:256] live in an even (Scalar) pair,
            # cols [256:512] in an odd (Vector) pair, for every core.
            HH = CH // 2

            def mk_ed_act(k):
                if ed_t[k] is None:
                    ed_t[k] = cp.tile([1, CH], FP8, name=f"ed{k}", tag=f"ed{k}")
                nc.scalar.activation(
                    out=ed_t[k][:, 0:HH], in_=fsqPS_t[k][:, 0:HH],
                    func=AF.Exp, scale=1.0 / TEMP,
                )

            def mk_ed_dve(k):
                if ed_t[k] is None:
                    ed_t[k] = cp.tile([1, CH], FP8, name=f"ed{k}", tag=f"ed{k}")
                nc.vector.tensor_scalar(
                    out=ed_t[k][:, HH:CH].bitcast(I8), in0=fsqPS_t[k][:, HH:CH],
                    scalar1=A_TRICK, scalar2=B_TRICK, op0=ALU.mult, op1=ALU.add,
                )

            def mk_e1(k, step):
                # e1 = ed * (minv*conf) + (conf - 1); mc/cm1 folded on host
                i0 = k * CH
                if step == 0:
                    t = cp.tile([1, CH], F32, name=f"dgt{k}", tag=f"dgt{k}")
                    nc.vector.tensor_mul(t, ed_t[k], s_mc[:, i0 : i0 + CH])
                    e1_t[k] = t
                else:
                    t2 = cp.tile([1, CH], F32, name=f"e1{k}", tag=f"e1{k}")
                    nc.vector.tensor_add(t2, e1_t[k], s_cm1[:, i0 : i0 + CH])
                    e1_t[k] = t2

            # ---------------- main pipeline --------------------------------
            exps_t = {}

            def raw_pair(k, p):
                rawPS = rp.tile([128, 2 * CH], F32, name=f"raw{k}_{p}", tag="raw")
                for q in (0, 1):
                    t = 2 * p + q
                    nc.tensor.matmul(
                        rawPS[:, q * CH : (q + 1) * CH],
                        lhsT=s_fTg[:, 128 * t : 128 * (t + 1)],
                        rhs=s_fTc[:, k * CH : (k + 1) * CH],
                        start=True,
                        stop=True,
                    )
                return rawPS

            def exp_pair(k, p, rawPS):
                exps = ep.tile([128, 2 * CH], FP8, name=f"exps{k}_{p}", tag="exps")
                if _is_act(p):
                    nc.scalar.activation(
                        out=exps, in_=rawPS, func=AF.Exp, scale=1.0 / TEMP
                    )
                else:
                    nc.vector.tensor_scalar(
                        out=exps[:].bitcast(I8), in0=rawPS,
                        scalar1=A_TRICK, scalar2=B_TRICK,
                        op0=ALU.mult, op1=ALU.add,
                    )
                exps_t[(k, p)] = exps

            EPS_t = [None, None]

            def e_mm(k, p):
                if EPS_t[k] is None:
                    EPS_t[k] = pp.tile([CP, CH], F32, name=f"EPS{k}", tag="EPS")
                exps = exps_t.pop((k, p))
                nc.tensor.matmul(
                    EPS_t[k],
                    lhsT=s_TAg[:, 224 * p : 224 * (p + 1)].rearrange(
                        "a (two c) -> a two c", two=2
                    ),
                    rhs=exps[:].rearrange("a (two n) -> a two n", two=2),
                    start=(p == 0),
                    stop=(p == PAIRS - 1),
                    perf_mode=DR,
                )

            srowPS_t = [None, None]

            def mk_w2e(k):
                i0 = k * CH
                w2e = cp.tile([C, CH], BF16, name=f"W2E{k}", tag=f"W2E{k}")
                nc.vector.tensor_mul(w2e, EPS_t[k][0:C, :], s_W2c[:, i0 : i0 + CH])
                return w2e

            def mk_srow(k, w2e):
                srowPS = sp.tile([1, CH], F32, name=f"srowPS{k}", tag="sm")
                nc.tensor.matmul(
                    srowPS, lhsT=s_ones_bf[0:C, :], rhs=w2e, start=True, stop=True
                )
                srowPS_t[k] = srowPS

            def mk_sall(k):
                nc.vector.tensor_sub(
                    s_Sall[:, k * CH : (k + 1) * CH], srowPS_t[k], e1_t[k]
                )

            lg_t = [None, None]

            def mk_ln(k):
                lg = cp.tile([1, CH], F32, name=f"lg{k}", tag=f"lg{k}")
                nc.scalar.activation(
                    out=lg, in_=s_Sall[:, k * CH : (k + 1) * CH], func=AF.Ln,
                    accum_out=outsb[:, k : k + 1],
                )
                lg_t[k] = lg

            # Side-work injected during the DMA-paced ramp (first ~14 pairs):
            vec_after = {
                (0, 3): [lambda: mk_ed_dve(0)],
                (0, 5): [lambda: mk_e1(0, 0)],
                (0, 7): [lambda: mk_e1(0, 1)],
                (0, 9): [lambda: mk_ed_dve(1)],
                (0, 11): [lambda: mk_e1(1, 0)],
                (0, 13): [lambda: mk_e1(1, 1)],
                (1, 6): [lambda: mk_sall(0)],
            }
            # Scalar-queue side-work
            sca_after = {
                (0, 3): [lambda: mk_ed_act(0)],
                (0, 8): [lambda: mk_ed_act(1)],
                (1, 9): [lambda: mk_ln(0)],
            }

            for k in (0, 1):
                raw_t = {}
                for p in range(PAIRS):
                    raw_t[p] = raw_pair(k, p)
                    if k == 0 and p == 2:
                        mk_fsq(0)
                    if k == 0 and p == 6:
                        mk_fsq(1)
                    if k == 1 and p == 3:
                        # chunk-0 tail reduction once W2E0 is ready
                        mk_srow(0, w2e0)
                    exp_pair(k, p, raw_t.pop(p))
                    for fn in sca_after.pop((k, p), ()):
                        fn()
                    for fn in vec_after.pop((k, p), ()):
                        fn()
                    if p >= 2:
                        e_mm(k, p - 2)
                e_mm(k, PAIRS - 2)
                e_mm(k, PAIRS - 1)
                if k == 0:
                    w2e0 = mk_w2e(0)
                else:
                    w2e1 = mk_w2e(1)
                    mk_srow(1, w2e1)

            # ---------------- tail: ln + accumulate ------------------------
            mk_sall(1)
            mk_ln(1)
            nc.sync.dma_start(out=outd[:], in_=outsb)

    nc.finalize()
    return nc


def _get_nc():
    if "nc" not in _NC_CACHE:
        _NC_CACHE["nc"] = _build_nc()
    return _NC_CACHE["nc"]


def _prep_inputs(centers1, features, targets, conf_mask):
    f32 = np.float32
    features = np.ascontiguousarray(features, dtype=f32)
    centers1 = np.ascontiguousarray(centers1, dtype=f32).reshape(-1, D)
    targets = np.ascontiguousarray(targets, dtype=f32)
    conf_mask = np.ascontiguousarray(conf_mask, dtype=f32)

    feats_all = np.concatenate([features, centers1], axis=0)  # [N, D]
    fa_pad = np.zeros((NPAD2, D), dtype=f32)
    fa_pad[:N] = feats_all
    fTg_np = np.ascontiguousarray(fa_pad.T).astype(BF)  # [D, NPAD2]

    TA_pad = np.zeros((NPAD2, CP), dtype=f32)
    TA_pad[:B2, :C] = targets
    TA_pad[B2 : B2 + C, :C] = np.eye(C, dtype=f32)
    TAg_np = np.ascontiguousarray(
        TA_pad.reshape(TILES, 128, CP).transpose(1, 0, 2).reshape(128, TILES * CP)
    ).astype(F8NP)

    labels = targets.argmax(axis=1)
    cc = targets.sum(axis=0, dtype=np.float64) + 1.0  # [C]
    mpos = np.maximum(cc - 1.0, 1.0)
    W2 = np.where(
        targets.T == 1.0, 1.0 / mpos[:, None], 1.0 / cc[:, None]
    )  # [C, B2] f64
    minv_all = (1.0 / mpos[labels]).astype(f32)  # [B2]

    # host linear term: exact f32-feature positive-pair mean logits
    gsum = np.zeros((C, D), dtype=np.float64)
    np.add.at(gsum, labels, features.astype(np.float64))
    gsum += centers1.astype(np.float64)  # class centers are their own class
    feats64 = features.astype(np.float64)
    Sm = (feats64 * gsum[labels]).sum(axis=1) - (feats64 * feats64).sum(axis=1)
    conf64 = conf_mask.astype(np.float64)
    numB = float((conf64 * (1.0 / TEMP) * Sm / mpos[labels]).sum())
    den = float(conf64.sum())

    in_maps = []
    for c in range(CORES):
        rows = slice(c * R, (c + 1) * R)
        fTc_np = np.ascontiguousarray(fTg_np[:, c * R : (c + 1) * R])
        W2c_np = np.ascontiguousarray(
            (W2[:, rows] * conf64[None, rows]).astype(f32)
        ).astype(BF)
        in_maps.append(
            {
                "fTg": fTg_np,
                "TAg": TAg_np,
                "fTc": fTc_np,
                "W2c": W2c_np,
                "mc": np.ascontiguousarray(
                    (minv_all[rows] * conf_mask[rows]).reshape(1, R)
                ),
                "cm1": np.ascontiguousarray(
                    (conf_mask[rows] - 1.0).reshape(1, R)
                ),
            }
        )
    return in_maps, numB, den


def _run(centers1, features, targets, conf_mask, trace=False, trace_cores=None):
    in_maps, numB, den = _prep_inputs(centers1, features, targets, conf_mask)
    nc = _get_nc()
    kwargs = {}
    if trace:
        # NTFF profiling under axon: shim the (absent) antenv.axon_hooks
        # module and skip the artifact bucket upload.
        import types
        import concourse.bass_utils as bass_utils

        if "antenv.axon_hooks" not in sys.modules:
            mod = types.ModuleType("antenv.axon_hooks")
            mod._hook = None

            def set_axon_ntff_profile_hook(h):
                mod._hook = h

            def get_axon_ntff_profile_hook():
                return mod._hook

            mod.set_axon_ntff_profile_hook = set_axon_ntff_profile_hook
            mod.get_axon_ntff_profile_hook = get_axon_ntff_profile_hook
            sys.modules["antenv.axon_hooks"] = mod
            from trn_agent_boot.trn_boot import _ntff_profile_via_ctypes

            set_axon_ntff_profile_hook(
                _ntff_profile_via_ctypes("/opt/axon/libaxon_pjrt.so")
            )
        bass_utils.upload_artifacts = lambda tmpdir: "local://" + tmpdir
        kwargs = {"trace": True}
        if trace_cores is not None:
            kwargs["trace_cores"] = trace_cores
    res = run_bass_kernel_spmd(nc, in_maps, core_ids=list(range(CORES)), **kwargs)
    numA = 0.0
    for r in res.results:
        numA += float(r["out"][0, 0]) + float(r["out"][0, 1])
    loss = np.array((numA - numB) / den, dtype=np.float32)
    return loss, res


def kernel(centers1, features, targets, cls_num_list, conf_mask):
    loss, _ = _run(centers1, features, targets, conf_mask)
    return loss
